# revision 21
# baseline (speedup 1.0000x reference)
"""Tensor-parallel decoder layer on 8 TRN2 NeuronCores.

Sharding / schedule:
  - Attention: 16 heads -> 2 per core. Per-core partial attn_out is
    ReduceScattered in fp16, chunked by row-QUARTER (so each RS starts
    while later attnv row-blocks still compute). Core c owns rows
    {512q + 64c .. 512q + 64c + 64 | q = 0..3}.
  - Only the ATTENTION part of y = x + bv + attn_out is AllGathered,
    transposed (x^T is already resident on every core): per fh-half,
    rsum = head0+head1 RS chunks (DMA-accumulated), DMA-xbar transposed,
    AllGathered. The fh=0 AllGather is issued before the last head's
    fh=1 ReduceScatter so it runs during attention compute.
  - Global LayerNorm stats (scalar mean/var over [S,E]): per-core
    partials from the transposed own-shard, tiny fp32 AllReduce issued
    before the second AllGather on the collective queue.
  - LN1 affine is applied redundantly per-core over the full y^T
    (h^T = (x^T + a^T)*rstd*g^T + (bv+...-m*rstd terms)), written IN
    PLACE over x^T, which then feeds FFN1 directly (no PE transposes).
  - FFN: hidden dim 8192 -> 1024 per core; FFN2 partial output
    ReduceScattered per (512-col, row-quarter) chunk fp16.

Matmul layout: PE computes out = lhsT.T @ rhs (contraction on the
partition dim). Scores are built transposed so exp(S^T) tiles feed
attn@v as lhsT; softmax normalization is deferred via a ones-column
rowsum matmul applied as a per-partition scale on the PSUM->SBUF copy.
All weights are pre-cast to bf16 on the host.
"""

import math
import sys

sys.path.insert(0, "/opt/trn_rl_repo")

import numpy as np
import ml_dtypes

_bf16 = ml_dtypes.bfloat16

import concourse.bass as bass
import concourse.mybir as mybir
import concourse.tile as tile
from concourse import bacc
from concourse.bass_utils import run_bass_kernel_spmd

S, E, H, KD, FF = 2048, 2048, 16, 128, 8192
EPS = 1e-5
NCORES = 8
HPC = H // NCORES          # heads per core = 2
FSH = FF // NCORES         # ffn hidden shard = 1024
RROWS = S // NCORES        # rows owned per core = 256 (4 chunks of 64)
QROWS = 64                 # rows per core per quarter
NTOT = float(S * E)
ISCALE = 1.0 / math.sqrt(KD)

F32 = mybir.dt.float32
BF16 = mybir.dt.bfloat16
F16 = mybir.dt.float16
AF = mybir.ActivationFunctionType
AL = mybir.AluOpType
AX = mybir.AxisListType

# packed triangular offsets for eT tiles: tile(tc, sb) at TRI[sb] + tc
TRI = [0, 4, 12, 24]
NTRI = 40


def _rows_idx(c):
    return np.concatenate([512 * q + QROWS * c + np.arange(QROWS) for q in range(4)])


def _build():
    nc = bacc.Bacc(
        "TRN2",
        target_bir_lowering=False,
        debug=False,
        enable_asserts=True,
        num_devices=NCORES,
    )

    # ---- external I/O (per-core shards prepared on the host) ----
    xtb_d = nc.dram_tensor("xtb", [128, 16, S], BF16, kind="ExternalInput")
    wq_d = nc.dram_tensor("wqt", [HPC, 128, 16, KD], BF16, kind="ExternalInput")
    wk_d = nc.dram_tensor("wkt", [HPC, 128, 16, KD], BF16, kind="ExternalInput")
    wv_d = nc.dram_tensor("wvt", [HPC, 128, 16, E], BF16, kind="ExternalInput")
    w1_d = nc.dram_tensor("w1t", [128, 8, 2048], BF16, kind="ExternalInput")
    w2_d = nc.dram_tensor("w2t", [128, 8, 4, 512], BF16, kind="ExternalInput")
    bq_d = nc.dram_tensor("bqs", [128, HPC], F32, kind="ExternalInput")
    bk_d = nc.dram_tensor("bks", [128, HPC], F32, kind="ExternalInput")
    b1_d = nc.dram_tensor("b1s", [128, 8], F32, kind="ExternalInput")
    yb1_d = nc.dram_tensor("yb1", [128, E], F32, kind="ExternalInput")
    yb1T_d = nc.dram_tensor("yb1T", [128, 16], F32, kind="ExternalInput")
    yb2_d = nc.dram_tensor("yb2", [128, E], F32, kind="ExternalInput")
    xr_d = nc.dram_tensor("xr", [RROWS, E], F32, kind="ExternalInput")
    xg_d = nc.dram_tensor("xg", [128, 16, RROWS], BF16, kind="ExternalInput")
    lng_d = nc.dram_tensor("lngr", [RROWS, E], BF16, kind="ExternalInput")
    lnb_d = nc.dram_tensor("lnbr", [RROWS, E], BF16, kind="ExternalInput")
    gT_d = nc.dram_tensor("gT", [128, 16, S], BF16, kind="ExternalInput")
    bT_d = nc.dram_tensor("bT", [128, 16, S], BF16, kind="ExternalInput")
    mask_d = nc.dram_tensor("mask", [128, 4, 512], BF16, kind="ExternalInput")
    ones_d = nc.dram_tensor("ones", [128, 8], F32, kind="ExternalInput")
    onesr_d = nc.dram_tensor("onesr", [1, 128], F32, kind="ExternalInput")
    out_d = nc.dram_tensor("out", [RROWS, E], F32, kind="ExternalOutput")

    RG = [list(range(NCORES))]

    with tile.TileContext(nc) as tc:
        with (
            tc.tile_pool(name="persist", bufs=1) as pp,
            tc.tile_pool(name="dram", bufs=1, space="DRAM") as dp,
            tc.tile_pool(name="ps512", bufs=4, space="PSUM") as ps512,
            tc.tile_pool(name="psT", bufs=2, space="PSUM") as psT,
            tc.tile_pool(name="psR", bufs=2, space="PSUM") as psR,
        ):
            # ---- collective bounce buffers (internal DRAM) ----
            att_in = [
                [
                    dp.tile([S, FSH], F16, name=f"att_in_{h}_{fh}", tag=f"ati{h}{fh}")
                    for fh in range(2)
                ]
                for h in range(HPC)
            ]
            att_out = [
                [
                    [
                        dp.tile(
                            [QROWS, FSH],
                            F16,
                            name=f"att_out_{h}_{fh}_{q}",
                            tag=f"ato{h}{fh}{q}",
                        )
                        for q in range(4)
                    ]
                    for fh in range(2)
                ]
                for h in range(HPC)
            ]
            st1_in = dp.tile([1, 8], F32, name="st1_in", tag="st1i")
            st1_out = dp.tile([1, 8], F32, name="st1_out", tag="st1o", addr_space="Shared")
            st2_in = dp.tile([1, 8], F32, name="st2_in", tag="st2i")
            st2_out = dp.tile([1, 8], F32, name="st2_out", tag="st2o", addr_space="Shared")
            agt_in = [
                dp.tile([E // 2, RROWS], F16, name=f"agt_in{j}", tag=f"agi{j}")
                for j in range(2)
            ]
            agt_out = [
                dp.tile(
                    [NCORES * (E // 2), RROWS],
                    F16,
                    name=f"agt_out{j}",
                    tag=f"ago{j}",
                    addr_space="Shared",
                )
                for j in range(2)
            ]
            ffn_in = [
                dp.tile([S, 512], F16, name=f"ffn_in_{eb}", tag=f"ffi{eb}")
                for eb in range(4)
            ]
            ffn_out = [
                [
                    dp.tile([QROWS, 512], F16, name=f"ffn_out_{eb}_{q}", tag=f"ffo{eb}{q}")
                    for q in range(4)
                ]
                for eb in range(4)
            ]

            # ---- persistent small tiles ----
            xh = pp.tile([128, 16, S], BF16, name="xh")  # x^T; becomes h^T in place
            onesc = pp.tile([128, 8], F32, name="onesc")
            nc.sync.dma_start(onesc[:], ones_d[:])
            onesr = pp.tile([1, 128], F32, name="onesr")
            nc.sync.dma_start(onesr[:], onesr_d[:])
            ones_bf = pp.tile([128, 1], BF16, name="ones_bf")
            nc.vector.tensor_copy(out=ones_bf[:], in_=onesc[:, 0:1])
            bq_sb = pp.tile([128, HPC], F32, name="bq_sb")
            nc.sync.dma_start(bq_sb[:], bq_d[:])
            bk_sb = pp.tile([128, HPC], F32, name="bk_sb")
            nc.sync.dma_start(bk_sb[:], bk_d[:])
            b1_sb = pp.tile([128, 8], F32, name="b1_sb")
            nc.sync.dma_start(b1_sb[:], b1_d[:])
            yb1T = pp.tile([128, 16], F32, name="yb1T")
            nc.sync.dma_start(yb1T[:], yb1T_d[:])
            recips = pp.tile([128, HPC, 16], F32, name="recips")
            bc1 = pp.tile([128, 2], F32, name="bc1")

            with tc.tile_pool(name="attn", bufs=1) as ap_:
                maskb = ap_.tile([128, 4, 512], BF16, name="maskb")
                eT = ap_.tile([128, NTRI, 512], BF16, name="eT")
                v_sb = ap_.tile([128, 16, 512], BF16, name="v_sb")

                with (
                    tc.tile_pool(name="wvb", bufs=2) as wvbp,
                    tc.tile_pool(name="astg", bufs=4) as astg,
                ):

                    def _vproj(h, fh, fb):
                        with nc.named_scope(f"vproj{h}{fh}{fb}"):
                            wvb = wvbp.tile([128, 16, 512], BF16, name="wvb", tag="wvb")
                            nc.sync.dma_start(
                                wvb[:],
                                wv_d[
                                    h, :, :,
                                    fh * 1024 + fb * 512 : fh * 1024 + (fb + 1) * 512,
                                ],
                            )
                            for tcn in range(16):
                                pv = ps512.tile([128, 512], F32, name="pv", tag="p512")
                                for eo in range(16):
                                    nc.tensor.matmul(
                                        pv[:],
                                        xh[:, eo, tcn * 128 : (tcn + 1) * 128],
                                        wvb[:, eo, :],
                                        start=(eo == 0),
                                        stop=(eo == 15),
                                    )
                                nc.vector.tensor_copy(out=v_sb[:, tcn, :], in_=pv[:])

                    def _attnv(h, fh, fb):
                        with nc.named_scope(f"attnv{h}{fh}{fb}"):
                            do_r = fh == 0 and fb == 0
                            for i in range(15, -1, -1):
                                sb, so = i // 4, (i % 4) * 128
                                pa = ps512.tile([128, 512], F32, name="pa", tag="p512")
                                if do_r:
                                    pr = psR.tile([128, 1], F32, name="pr", tag="pr")
                                for tcn in range(i + 1):
                                    lhs = eT[:, TRI[sb] + tcn, so : so + 128]
                                    nc.tensor.matmul(
                                        pa[:],
                                        lhs,
                                        v_sb[:, tcn, :],
                                        start=(tcn == 0),
                                        stop=(tcn == i),
                                    )
                                    if do_r:
                                        nc.tensor.matmul(
                                            pr[:],
                                            lhs,
                                            ones_bf[:],
                                            start=(tcn == 0),
                                            stop=(tcn == i),
                                        )
                                if do_r:
                                    rsf = astg.tile([128, 1], F32, name="rsf", tag="rsf")
                                    nc.vector.tensor_copy(out=rsf[:], in_=pr[:])
                                    nc.vector.reciprocal(recips[:, h, i : i + 1], rsf[:])
                                stg = astg.tile([128, 512], F16, name="stg", tag="stg")
                                nc.scalar.activation(
                                    stg[:],
                                    pa[:],
                                    AF.Copy,
                                    scale=recips[:, h, i : i + 1],
                                )
                                nc.sync.dma_start(
                                    att_in[h][fh][
                                        i * 128 : (i + 1) * 128,
                                        fb * 512 : (fb + 1) * 512,
                                    ],
                                    stg[:],
                                )
                                if fb == 1 and i % 4 == 0:
                                    q = i // 4
                                    nc.gpsimd.collective_compute(
                                        "ReduceScatter",
                                        AL.add,
                                        replica_groups=RG,
                                        ins=[att_in[h][fh][512 * q : 512 * (q + 1), :]],
                                        outs=[att_out[h][fh][q][:]],
                                    )

                    def _scores(h):
                        with nc.named_scope(f"scores{h}"):
                            for sb in range(4):
                                for tcn in range(4 * sb + 4):
                                    psc = ps512.tile([128, 512], F32, name="psc", tag="p512")
                                    nc.tensor.matmul(
                                        psc[:],
                                        qkT[:, 1, h, tcn * 128 : (tcn + 1) * 128],
                                        qkT[:, 0, h, sb * 512 : (sb + 1) * 512],
                                        start=True,
                                        stop=True,
                                    )
                                    dst = eT[:, TRI[sb] + tcn, :]
                                    if tcn >= 4 * sb:
                                        etmp = astg.tile(
                                            [128, 512], BF16, name="etmp", tag="etmp", bufs=3
                                        )
                                        nc.scalar.activation(etmp[:], psc[:], AF.Exp)
                                        nc.vector.tensor_tensor(
                                            dst, etmp[:], maskb[:, tcn - 4 * sb, :], AL.mult
                                        )
                                    else:
                                        nc.scalar.activation(dst, psc[:], AF.Exp)

                    def _midwork(fh):
                        with nc.named_scope(f"mid{fh}"):
                            aTo = mw.tile(
                                [128, 8, RROWS], F16, name="aTo", tag="aTo", bufs=2
                            )
                            for rt in range(2):
                                for qh in range(2):
                                    q = 2 * rt + qh
                                    for h in range(2):
                                        nc.sync.dma_start(
                                            rof[QROWS * qh : QROWS * (qh + 1), rt, h, :],
                                            att_out[h][fh][q][:],
                                        )
                                nc.vector.tensor_tensor(
                                    rsum[:, rt, :], rof[:, rt, 0, :], rof[:, rt, 1, :], AL.add
                                )
                                nc.sync.dma_start(
                                    aTo[:, :, rt * 128 : (rt + 1) * 128],
                                    rsum[:, rt, :],
                                    transpose=True,
                                )
                            nc.sync.dma_start(
                                agt_in[fh].rearrange("(ec p) s -> p ec s", p=128),
                                aTo[:],
                            )
                            if fh == 0:
                                nc.gpsimd.collective_compute(
                                    "AllGather",
                                    AL.bypass,
                                    replica_groups=RG,
                                    ins=[agt_in[0][:]],
                                    outs=[agt_out[0][:]],
                                )
                            # LN1 stats partial from transposed own shard
                            nc.vector.tensor_tensor(
                                yo[:], xg[:, fh * 8 : (fh + 1) * 8, :], aTo[:], AL.add
                            )
                            yof = yo.rearrange("p a b -> p (a b)")
                            nc.vector.tensor_reduce(
                                parts[:, fh : fh + 1], yof, axis=AX.X, op=AL.add
                            )
                            nc.scalar.activation(
                                sqs2[:], yof, AF.Square,
                                accum_out=parts[:, 2 + fh : 3 + fh],
                            )

                    with tc.tile_pool(name="qkp", bufs=1) as qkp:
                        qkT = qkp.tile([128, 2, HPC, S], BF16, name="qkT")
                        with tc.tile_pool(name="qkw", bufs=1) as qkw:
                            wqk = [
                                [
                                    qkw.tile(
                                        [128, 16, KD], BF16,
                                        name=f"wqk{h}{qi}", tag=f"wqk{h}{qi}",
                                    )
                                    for qi in range(2)
                                ]
                                for h in range(HPC)
                            ]
                            for h in range(HPC):
                                nc.sync.dma_start(wqk[h][0][:], wq_d[h])
                                nc.sync.dma_start(wqk[h][1][:], wk_d[h])
                            for eo in range(16):
                                nc.sync.dma_start(xh[:, eo, :], xtb_d[:, eo, :])
                            nc.sync.dma_start(maskb[:], mask_d[:])
                            with tc.tile_pool(name="prep", bufs=1) as prep, nc.named_scope("prep"):
                                wtile = prep.tile([128, 512], BF16, name="wtile", tag="wtile")
                                nc.vector.memset(wtile[:], 0.0)
                                for _w in range(24):
                                    pw = ps512.tile([128, 512], F32, name="pw", tag="p512")
                                    nc.tensor.matmul(
                                        pw[:], wtile[:, :128], wtile[:], start=True, stop=True
                                    )
                            with nc.named_scope("qkproj"):
                                for h in range(HPC):
                                    for qi, (b_sb, scl) in enumerate(
                                        ((bq_sb, ISCALE), (bk_sb, 1.0))
                                    ):
                                        wb = wqk[h][qi]
                                        pqs = [
                                            ps512.tile([128, 512], F32, name=f"pq{sb}", tag="p512")
                                            for sb in range(4)
                                        ]
                                        for eo in range(16):
                                            for sb in range(4):
                                                nc.tensor.matmul(
                                                    pqs[sb][:],
                                                    wb[:, eo, :],
                                                    xh[:, eo, sb * 512 : (sb + 1) * 512],
                                                    start=(eo == 0),
                                                    stop=(eo == 15),
                                                )
                                        for sb in range(4):
                                            nc.scalar.activation(
                                                qkT[:, qi, h, sb * 512 : (sb + 1) * 512],
                                                pqs[sb][:],
                                                AF.Identity,
                                                bias=b_sb[:, h : h + 1],
                                                scale=scl,
                                            )
                        # head 0 fully inside qkp (eT reused by head 1)
                        _scores(0)
                        for fh in range(2):
                            for fb in range(2):
                                _vproj(0, fh, fb)
                                _attnv(0, fh, fb)
                        _scores(1)
                    # qkp closed: its space is reused by the midwork pool
                    with tc.tile_pool(name="mw", bufs=1) as mw:
                      xg = mw.tile([128, 16, RROWS], BF16, name="xg")
                      nc.sync.dma_start(xg[:], xg_d[:])
                      rof = mw.tile([128, 2, 2, FSH], F16, name="rof")
                      rsum = mw.tile([128, 2, FSH], F16, name="rsum")
                      yo = mw.tile([128, 8, RROWS], BF16, name="yo")
                      sqs2 = mw.tile([128, 8 * RROWS], BF16, name="sqs2")
                      parts = mw.tile([128, 4], F32, name="parts")
                      stat1 = mw.tile([1, 16], F32, name="stat1")
                      for fb in range(2):
                        _vproj(1, 0, fb)
                        _attnv(1, 0, fb)
                      _midwork(0)
                      for fb in range(2):
                        _vproj(1, 1, fb)
                        _attnv(1, 1, fb)
                      _midwork(1)
                      # LN1 stats AllReduce (before the 2nd AllGather on the
                      # collective queue)
                      with nc.named_scope("ln1ar"):
                        pstat = psT.tile([128, 128], F32, name="pstat", tag="pt")
                        nc.tensor.matmul(
                            pstat[:1, :4], onesc[:, 0:1], parts[:], start=True, stop=True
                        )
                        nc.vector.tensor_copy(out=stat1[:, 0:4], in_=pstat[:1, :4])
                        nc.vector.memset(stat1[:, 8:16], 0.0)
                        nc.vector.tensor_reduce(
                            stat1[:, 8:9], stat1[:, 0:2], axis=AX.X, op=AL.add
                        )
                        nc.vector.tensor_reduce(
                            stat1[:, 9:10], stat1[:, 2:4], axis=AX.X, op=AL.add
                        )
                        nc.sync.dma_start(st1_in[:], stat1[:, 8:16])
                        nc.gpsimd.collective_compute(
                            "AllReduce", AL.add, replica_groups=RG,
                            ins=[st1_in[:]], outs=[st1_out[:]],
                        )
                      nc.gpsimd.collective_compute(
                          "AllGather",
                          AL.bypass,
                          replica_groups=RG,
                          ins=[agt_in[1][:]],
                          outs=[agt_out[1][:]],
                      )
                      with nc.named_scope("ln1sc"):
                          bcv = _ln_scalars(nc, mw, psT, onesr, st1_out)
                          nc.vector.tensor_copy(out=bc1[:], in_=bcv[:])

            # =========== LN1 affine over full y^T, in place into xh ===========
            with tc.tile_pool(name="aff", bufs=1) as gb, nc.named_scope("affine"):
                bias16 = gb.tile([128, 16], F32, name="bias16")
                # bias per (e-partition, eo): (yb1T - m) * rstd
                nc.scalar.activation(
                    bias16[:], yb1T[:], AF.Identity, bias=bc1[:, 0:1], scale=bc1[:, 1:2]
                )
                for eo in range(16):
                    fh, el = eo // 8, eo % 8
                    aTe = gb.tile([128, S], F16, name="aTe", tag="aTe", bufs=3)
                    for c in range(NCORES):
                        nc.sync.dma_start(
                            aTe[:, c * RROWS : (c + 1) * RROWS],
                            agt_out[fh][
                                c * (E // 2) + el * 128 : c * (E // 2) + (el + 1) * 128, :
                            ],
                        )
                    gch = gb.tile([128, S], BF16, name="gch", tag="gch", bufs=3)
                    nc.sync.dma_start(gch[:], gT_d[:, eo, :])
                    bch = gb.tile([128, S], BF16, name="bch", tag="bch", bufs=3)
                    nc.sync.dma_start(bch[:], bT_d[:, eo, :])
                    t1 = gb.tile([128, S], BF16, name="t1", tag="t1", bufs=2)
                    # xh's s-axis is global order; aTe/t1 are in pi-order
                    # (core-major own-row order): pi(256c + 64q + j) =
                    # 512q + 64c + j. Read xh permuted, per q-group.
                    xhv = xh[:, eo, :].rearrange("p (q c j) -> p q c j", q=4, c=8)
                    t1v = t1.rearrange("p (c q j) -> p c q j", c=8, q=4)
                    aev = aTe.rearrange("p (c q j) -> p c q j", c=8, q=4)
                    for qq in range(4):
                        nc.vector.tensor_tensor(
                            t1v[:, :, qq, :], xhv[:, qq, :, :], aev[:, :, qq, :], AL.add
                        )
                    t2 = gb.tile([128, S], BF16, name="t2", tag="t2", bufs=2)
                    nc.scalar.activation(
                        t2[:], t1[:], AF.Identity,
                        bias=bias16[:, eo : eo + 1], scale=bc1[:, 1:2],
                    )
                    nc.vector.tensor_tensor(t2[:], t2[:], gch[:], AL.mult)
                    nc.vector.tensor_tensor(xh[:, eo, :], t2[:], bch[:], AL.add)

            # =========== FFN + rowwise h_own + LN2 ===========
            with tc.tile_pool(name="ffn", bufs=1) as fp:
                zT = fp.tile([128, 8, S], BF16, name="zT")
                h_own = fp.tile([128, 2, E], BF16, name="h_own")
                ys = fp.tile([128, 2, E], F32, name="ys")
                lngt = fp.tile([128, 2, E], BF16, name="lngt")
                lnbt = fp.tile([128, 2, E], BF16, name="lnbt")

                with nc.named_scope("ffn1"), tc.tile_pool(name="wst", bufs=2) as wst:
                    for ft in range(8):
                        w1b = wst.tile([128, 2048], BF16, name="w1b", tag="w1b")
                        nc.sync.dma_start(w1b[:], w1_d[:, ft, :])
                        pzs = [
                            ps512.tile([128, 512], F32, name=f"pz{sb}", tag="p512")
                            for sb in range(4)
                        ]
                        for eo in range(16):
                            for sb in range(4):
                                nc.tensor.matmul(
                                    pzs[sb][:],
                                    w1b[:, eo * 128 : (eo + 1) * 128],
                                    xh[:, eo, sb * 512 : (sb + 1) * 512],
                                    start=(eo == 0),
                                    stop=(eo == 15),
                                )
                        for sb in range(4):
                            nc.scalar.activation(
                                zT[:, ft, sb * 512 : (sb + 1) * 512],
                                pzs[sb][:],
                                AF.Relu,
                                bias=b1_sb[:, ft : ft + 1],
                            )

                # rowwise y rebuild -> h_own (off the critical path; needs bc1)
                with nc.named_scope("hown"), tc.tile_pool(name="hop", bufs=1) as hp:
                    yb1t = hp.tile([128, E], F32, name="yb1t")
                    nc.sync.dma_start(yb1t[:], yb1_d[:])
                    nc.sync.dma_start(lngt[:], lng_d.ap().rearrange("(t p) e -> p t e", p=128))
                    nc.sync.dma_start(lnbt[:], lnb_d.ap().rearrange("(t p) e -> p t e", p=128))
                    for rt in range(2):
                        xrt = hp.tile([128, E], F32, name="xrt", tag="xrt")
                        nc.sync.dma_start(xrt[:], xr_d[rt * 128 : (rt + 1) * 128, :])
                        nc.vector.tensor_tensor(ys[:, rt, :], xrt[:], yb1t[:], AL.add)
                        for fh in range(2):
                            dstv = ys[:, rt, fh * FSH : (fh + 1) * FSH]
                            for h in range(2):
                                rof2 = hp.tile([128, FSH], F16, name="rof2", tag="rof2", bufs=2)
                                for qh in range(2):
                                    q = 2 * rt + qh
                                    nc.sync.dma_start(
                                        rof2[QROWS * qh : QROWS * (qh + 1), :],
                                        att_out[h][fh][q][:],
                                    )
                                nc.vector.tensor_tensor(dstv, dstv, rof2[:], AL.add)
                        ht = hp.tile([128, E], F32, name="ht", tag="ht")
                        nc.scalar.activation(
                            ht[:], ys[:, rt, :], AF.Identity,
                            bias=bc1[:, 0:1], scale=bc1[:, 1:2],
                        )
                        nc.vector.tensor_tensor(ht[:], ht[:], lngt[:, rt, :], AL.mult)
                        nc.vector.tensor_tensor(h_own[:, rt, :], ht[:], lnbt[:, rt, :], AL.add)

                with nc.named_scope("ffn2"), tc.tile_pool(name="w2p", bufs=2) as w2p:
                    # zT's s-axis is pi-ordered (follows hT): position
                    # P = 256c + 64q + j holds global row 512q + 64c + j.
                    # Stage each 128-row block into ffn_in at the row
                    # positions 512q + 64c + j so that the row-quarter
                    # ReduceScatter hands core c exactly its own rows.
                    # Odd i blocks fill quarters 2,3; even blocks 0,1.
                    iorder = [15, 13, 11, 9, 7, 5, 3, 1, 14, 12, 10, 8, 6, 4, 2, 0]
                    for eb in range(4):
                        w2b = w2p.tile([128, 8, 512], BF16, name="w2b", tag="w2b")
                        nc.sync.dma_start(w2b[:], w2_d[:, :, eb, :])
                        fv = ffn_in[eb].rearrange("(q c j) f -> q c j f", q=4, c=8)
                        for i in iorder:
                            pf = ps512.tile([128, 512], F32, name="pf", tag="p512")
                            for fc in range(8):
                                nc.tensor.matmul(
                                    pf[:],
                                    zT[:, fc, i * 128 : (i + 1) * 128],
                                    w2b[:, fc, :],
                                    start=(fc == 0),
                                    stop=(fc == 7),
                                )
                            fstg = w2p.tile([128, 512], F16, name="fstg", tag="fstg", bufs=4)
                            nc.scalar.activation(fstg[:], pf[:], AF.Copy)
                            q0 = 2 * (i % 2)
                            nc.sync.dma_start(
                                fv[q0 : q0 + 2, i // 2, :, :], fstg[:]
                            )
                            if i == 1:
                                for q in (3, 2):
                                    nc.gpsimd.collective_compute(
                                        "ReduceScatter",
                                        AL.add,
                                        replica_groups=RG,
                                        ins=[ffn_in[eb][512 * q : 512 * (q + 1), :]],
                                        outs=[ffn_out[eb][q][:]],
                                    )
                            if i == 0:
                                for q in (1, 0):
                                    nc.gpsimd.collective_compute(
                                        "ReduceScatter",
                                        AL.add,
                                        replica_groups=RG,
                                        ins=[ffn_in[eb][512 * q : 512 * (q + 1), :]],
                                        outs=[ffn_out[eb][q][:]],
                                    )

                # =========== LN2 + output ===========
                with tc.tile_pool(name="ln2", bufs=1) as l2, nc.named_scope("ln2"):
                    yb2t = l2.tile([128, E], F32, name="yb2t")
                    nc.sync.dma_start(yb2t[:], yb2_d[:])
                    for rt in range(2):
                        nc.vector.tensor_tensor(
                            ys[:, rt, :], h_own[:, rt, :], yb2t[:], AL.add
                        )
                        for eb in range(4):
                            fot = l2.tile([128, 512], F16, name="fot", tag="fot", bufs=2)
                            for qh in range(2):
                                q = 2 * rt + qh
                                nc.sync.dma_start(
                                    fot[QROWS * qh : QROWS * (qh + 1), :],
                                    ffn_out[eb][q][:],
                                )
                            dstv = ys[:, rt, eb * 512 : (eb + 1) * 512]
                            nc.vector.tensor_tensor(dstv, dstv, fot[:], AL.add)

                    _stats_ln(nc, tc, l2, psT, ys, onesc, onesr, st2_in, st2_out, RG)
                    bc2 = _ln_scalars(nc, l2, psT, onesr, st2_out)
                    for rt in range(2):
                        ot = l2.tile([128, E], F32, name="ot", tag="ot", bufs=2)
                        nc.scalar.activation(
                            ot[:],
                            ys[:, rt, :],
                            AF.Identity,
                            bias=bc2[:, 0:1],
                            scale=bc2[:, 1:2],
                        )
                        nc.vector.tensor_tensor(ot[:], ot[:], lngt[:, rt, :], AL.mult)
                        nc.vector.tensor_tensor(ot[:], ot[:], lnbt[:, rt, :], AL.add)
                        nc.sync.dma_start(out_d[rt * 128 : (rt + 1) * 128, :], ot[:])

    nc.compile()
    return nc


def _stats_ln(nc, tc, pool, psT, ys, onesc, onesr, st_in, st_out, RG):
    """partial sum/sumsq of ys [128, 2, E] -> tiny fp32 AllReduce."""
    parts = pool.tile([128, 8], F32, name="parts", tag="parts")
    sqs = pool.tile([128, E // 2], BF16, name="sqs", tag="sqs")
    for rt in range(2):
        for ch in range(2):
            idx = rt * 2 + ch
            ysl = ys[:, rt, ch * (E // 2) : (ch + 1) * (E // 2)]
            nc.vector.tensor_reduce(parts[:, idx : idx + 1], ysl, axis=AX.X, op=AL.add)
            nc.scalar.activation(
                sqs[:], ysl, AF.Square, accum_out=parts[:, 4 + idx : 5 + idx]
            )
    pstat = psT.tile([128, 128], F32, name="pstat", tag="pt")
    nc.tensor.matmul(pstat[:1, :8], onesc[:, 0:1], parts[:], start=True, stop=True)
    st4s = pool.tile([1, 8], F32, name="st4s", tag="st4s")
    nc.vector.tensor_copy(out=st4s[:], in_=pstat[:1, :8])
    st4 = pool.tile([1, 8], F32, name="st4", tag="st4")
    nc.vector.memset(st4[:], 0.0)
    nc.vector.tensor_reduce(st4[:, 0:1], st4s[:, 0:4], axis=AX.X, op=AL.add)
    nc.vector.tensor_reduce(st4[:, 1:2], st4s[:, 4:8], axis=AX.X, op=AL.add)
    nc.sync.dma_start(st_in[:], st4[:])
    nc.gpsimd.collective_compute(
        "AllReduce", AL.add, replica_groups=RG, ins=[st_in[:]], outs=[st_out[:]]
    )


def _ln_scalars(nc, pool, psT, onesr, st_out):
    """AllReduced (sum, sumsq) -> bc [128, 2] = (-m*rstd, rstd) broadcast."""
    so = pool.tile([1, 8], F32, name="so", tag="so")
    nc.sync.dma_start(so[:], st_out[:])
    sc = pool.tile([1, 8], F32, name="sc", tag="sc")
    nc.scalar.mul(sc[:, 0:1], so[:, 0:1], 1.0 / NTOT)
    nc.scalar.mul(sc[:, 1:2], so[:, 1:2], 1.0 / NTOT)
    nc.scalar.activation(sc[:, 2:3], sc[:, 0:1], AF.Square)
    nc.vector.tensor_tensor(sc[:, 3:4], sc[:, 1:2], sc[:, 2:3], AL.subtract)
    nc.vector.tensor_scalar_add(sc[:, 2:3], sc[:, 3:4], EPS)  # var + eps
    nc.scalar.activation(sc[:, 6:7], sc[:, 2:3], AF.Ln)
    nc.scalar.activation(sc[:, 4:5], sc[:, 6:7], AF.Exp, scale=-0.5)
    nc.vector.tensor_tensor(sc[:, 7:8], sc[:, 0:1], sc[:, 4:5], AL.mult)
    nc.scalar.mul(sc[:, 5:6], sc[:, 7:8], -1.0)
    s2 = pool.tile([1, 2], F32, name="s2", tag="s2")
    nc.vector.tensor_copy(out=s2[:, 0:1], in_=sc[:, 5:6])
    nc.vector.tensor_copy(out=s2[:, 1:2], in_=sc[:, 4:5])
    pb = psT.tile([128, 128], F32, name="pb", tag="pt")
    nc.tensor.matmul(pb[:, :2], onesr[:], s2[:], start=True, stop=True)
    bc = pool.tile([128, 2], F32, name="bc", tag="bc")
    nc.vector.tensor_copy(out=bc[:], in_=pb[:, :2])
    return bc


_NC_CACHE = None


def _get_nc():
    global _NC_CACHE
    if _NC_CACHE is None:
        _NC_CACHE = _build()
    return _NC_CACHE


def _shared_prep(inputs):
    """Host tensors identical across cores (built once)."""
    f32 = np.float32
    x = np.ascontiguousarray(inputs["input"], dtype=f32)
    ln_g, ln_b = inputs["ln_g"], inputs["ln_b"]
    bv, b2 = inputs["bv"], inputs["b2"]
    jj, tp, sf = np.meshgrid(
        np.arange(4), np.arange(128), np.arange(512), indexing="ij"
    )
    mask = ((128 * jj + tp) <= sf).astype(_bf16)
    xtb = np.ascontiguousarray(
        x.T.reshape(16, 128, S).transpose(1, 0, 2).astype(_bf16)
    )
    # h^T's s-axis is pi-ordered (core-major own-row order)
    pi = np.concatenate([_rows_idx(c) for c in range(NCORES)])
    gT = np.ascontiguousarray(
        np.asarray(ln_g, f32)[pi].T.reshape(16, 128, S).transpose(1, 0, 2).astype(_bf16)
    )
    bT = np.ascontiguousarray(
        np.asarray(ln_b, f32)[pi].T.reshape(16, 128, S).transpose(1, 0, 2).astype(_bf16)
    )
    bvsum = np.asarray(bv, f32).sum(axis=0)
    return {
        "x": x,
        "xtb": xtb,
        "gT": gT,
        "bT": bT,
        "mask": np.ascontiguousarray(mask.transpose(1, 0, 2)),
        "ones": np.ones((128, 8), dtype=f32),
        "onesr": np.ones((1, 128), dtype=f32),
        "yb1": np.ascontiguousarray(np.broadcast_to(bvsum, (128, E)), dtype=f32),
        "yb1T": np.ascontiguousarray(bvsum.reshape(16, 128).T, dtype=f32),
        "yb2": np.ascontiguousarray(
            np.broadcast_to(np.asarray(b2, f32), (128, E)), dtype=f32
        ),
        "bvsum": bvsum,
    }


def _prep_core(c, inputs, shared=None):
    f32 = np.float32
    if shared is None:
        shared = _shared_prep(inputs)
    x = shared["x"]
    Wq, Wk, Wv = inputs["Wq"], inputs["Wk"], inputs["Wv"]
    bq, bk = inputs["bq"], inputs["bk"]
    W1, b1, W2 = inputs["W1"], inputs["b1"], inputs["W2"]
    ln_g, ln_b = inputs["ln_g"], inputs["ln_b"]
    h0 = c * HPC
    wqt = np.ascontiguousarray(
        np.stack(
            [Wq[h0 + h].reshape(16, 128, KD).transpose(1, 0, 2) for h in range(HPC)]
        ).astype(_bf16)
    )
    wkt = np.ascontiguousarray(
        np.stack(
            [Wk[h0 + h].reshape(16, 128, KD).transpose(1, 0, 2) for h in range(HPC)]
        ).astype(_bf16)
    )
    wvt = np.ascontiguousarray(
        np.stack(
            [Wv[h0 + h].reshape(16, 128, E).transpose(1, 0, 2) for h in range(HPC)]
        ).astype(_bf16)
    )
    W1s = np.asarray(W1)[:, c * FSH : (c + 1) * FSH]
    w1t = np.ascontiguousarray(
        W1s.reshape(16, 128, 8, 128).transpose(1, 2, 0, 3).reshape(128, 8, 2048).astype(_bf16)
    )
    W2s = np.asarray(W2)[c * FSH : (c + 1) * FSH, :]
    w2t = np.ascontiguousarray(
        W2s.reshape(8, 128, 4, 512).transpose(1, 0, 2, 3).astype(_bf16)
    )
    bqs = np.ascontiguousarray((np.asarray(bq, f32)[h0 : h0 + HPC] * ISCALE).T, dtype=f32)
    bks = np.ascontiguousarray(np.asarray(bk, f32)[h0 : h0 + HPC].T, dtype=f32)
    b1s = np.ascontiguousarray(
        np.asarray(b1, f32)[c * FSH : (c + 1) * FSH].reshape(8, 128).T, dtype=f32
    )
    ridx = _rows_idx(c)
    xo = x[ridx] + shared["bvsum"][None, :]
    xg = np.ascontiguousarray(
        xo.T.reshape(16, 128, RROWS).transpose(1, 0, 2).astype(_bf16)
    )
    return {
        "xtb": shared["xtb"],
        "gT": shared["gT"],
        "bT": shared["bT"],
        "mask": shared["mask"],
        "ones": shared["ones"],
        "onesr": shared["onesr"],
        "yb1": shared["yb1"],
        "yb1T": shared["yb1T"],
        "yb2": shared["yb2"],
        "wqt": wqt,
        "wkt": wkt,
        "wvt": wvt,
        "w1t": w1t,
        "w2t": w2t,
        "bqs": bqs,
        "bks": bks,
        "b1s": b1s,
        "xr": np.ascontiguousarray(x[ridx], dtype=f32),
        "xg": xg,
        "lngr": np.ascontiguousarray(np.asarray(ln_g, f32)[ridx].astype(_bf16)),
        "lnbr": np.ascontiguousarray(np.asarray(ln_b, f32)[ridx].astype(_bf16)),
    }


def _assemble(results):
    out = np.empty((S, E), dtype=np.float32)
    for c in range(NCORES):
        out[_rows_idx(c)] = results[c]["out"]
    return out


def kernel(**inputs):
    nc = _get_nc()
    inputs = {k: np.asarray(v, dtype=np.float32) for k, v in inputs.items()}
    shared = _shared_prep(inputs)
    in_maps = [_prep_core(c, inputs, shared) for c in range(NCORES)]
    res = run_bass_kernel_spmd(nc, in_maps, core_ids=list(range(NCORES)))
    return np.ascontiguousarray(_assemble(res.results), dtype=np.float32)


# revision 22
# speedup vs baseline: 1.1381x; 1.1381x over previous
"""Tensor-parallel decoder layer on 8 TRN2 NeuronCores.

Sharding / schedule:
  - Attention: 16 heads -> 2 per core. Per-core partial attn_out is
    ReduceScattered in fp16, chunked by row-QUARTER (so each RS starts
    while later attnv row-blocks still compute). Core c owns rows
    {512q + 64c .. 512q + 64c + 64 | q = 0..3}.
  - Only the ATTENTION part of y = x + bv + attn_out is AllGathered,
    transposed (x^T is already resident on every core): per fh-half,
    rsum = head0+head1 RS chunks (DMA-accumulated), DMA-xbar transposed,
    AllGathered. The fh=0 AllGather is issued before the last head's
    fh=1 ReduceScatter so it runs during attention compute.
  - Global LayerNorm stats (scalar mean/var over [S,E]): per-core
    partials from the transposed own-shard, tiny fp32 AllReduce issued
    before the second AllGather on the collective queue.
  - LN1 affine is applied redundantly per-core over the full y^T
    (h^T = (x^T + a^T)*rstd*g^T + (bv+...-m*rstd terms)), written IN
    PLACE over x^T, which then feeds FFN1 directly (no PE transposes).
  - FFN: hidden dim 8192 -> 1024 per core; FFN2 partial output
    ReduceScattered per (512-col, row-quarter) chunk fp16.

Matmul layout: PE computes out = lhsT.T @ rhs (contraction on the
partition dim). Scores are built transposed so exp(S^T) tiles feed
attn@v as lhsT; softmax normalization is deferred via a ones-column
rowsum matmul applied as a per-partition scale on the PSUM->SBUF copy.
All weights are pre-cast to bf16 on the host.
"""

import math
import sys

sys.path.insert(0, "/opt/trn_rl_repo")

import numpy as np
import ml_dtypes

_bf16 = ml_dtypes.bfloat16

import concourse.bass as bass
import concourse.mybir as mybir
import concourse.tile as tile
from concourse import bacc
from concourse.bass_utils import run_bass_kernel_spmd

S, E, H, KD, FF = 2048, 2048, 16, 128, 8192
EPS = 1e-5
NCORES = 8
HPC = H // NCORES          # heads per core = 2
FSH = FF // NCORES         # ffn hidden shard = 1024
RROWS = S // NCORES        # rows owned per core = 256 (4 chunks of 64)
QROWS = 64                 # rows per core per quarter
NTOT = float(S * E)
ISCALE = 1.0 / math.sqrt(KD)

F32 = mybir.dt.float32
BF16 = mybir.dt.bfloat16
F16 = mybir.dt.float16
AF = mybir.ActivationFunctionType
AL = mybir.AluOpType
AX = mybir.AxisListType

# packed triangular offsets for eT tiles: tile(tc, sb) at TRI[sb] + tc
TRI = [0, 4, 12, 24]
NTRI = 40


def _rows_idx(c):
    return c * RROWS + np.arange(RROWS)


def _build():
    nc = bacc.Bacc(
        "TRN2",
        target_bir_lowering=False,
        debug=False,
        enable_asserts=True,
        num_devices=NCORES,
    )

    # ---- external I/O (per-core shards prepared on the host) ----
    xtb_d = nc.dram_tensor("xtb", [128, 16, S], BF16, kind="ExternalInput")
    wq_d = nc.dram_tensor("wqt", [HPC, 128, 16, KD], BF16, kind="ExternalInput")
    wk_d = nc.dram_tensor("wkt", [HPC, 128, 16, KD], BF16, kind="ExternalInput")
    wv_d = nc.dram_tensor("wvt", [HPC, 128, 16, E], BF16, kind="ExternalInput")
    w1_d = nc.dram_tensor("w1t", [128, 8, 2048], BF16, kind="ExternalInput")
    w2_d = nc.dram_tensor("w2t", [128, 8, 4, 512], BF16, kind="ExternalInput")
    bq_d = nc.dram_tensor("bqs", [128, HPC], F32, kind="ExternalInput")
    bk_d = nc.dram_tensor("bks", [128, HPC], F32, kind="ExternalInput")
    b1_d = nc.dram_tensor("b1s", [128, 8], F32, kind="ExternalInput")
    yb1_d = nc.dram_tensor("yb1", [128, E], F32, kind="ExternalInput")
    yb1T_d = nc.dram_tensor("yb1T", [128, 16], F32, kind="ExternalInput")
    yb2_d = nc.dram_tensor("yb2", [128, E], F32, kind="ExternalInput")
    xr_d = nc.dram_tensor("xr", [RROWS, E], F32, kind="ExternalInput")
    xg_d = nc.dram_tensor("xg", [128, 16, RROWS], BF16, kind="ExternalInput")
    lng_d = nc.dram_tensor("lngr", [RROWS, E], BF16, kind="ExternalInput")
    lnb_d = nc.dram_tensor("lnbr", [RROWS, E], BF16, kind="ExternalInput")
    gT_d = nc.dram_tensor("gT", [128, 16, S], BF16, kind="ExternalInput")
    bT_d = nc.dram_tensor("bT", [128, 16, S], BF16, kind="ExternalInput")
    mask_d = nc.dram_tensor("mask", [128, 4, 512], BF16, kind="ExternalInput")
    ones_d = nc.dram_tensor("ones", [128, 8], F32, kind="ExternalInput")
    onesr_d = nc.dram_tensor("onesr", [1, 128], F32, kind="ExternalInput")
    out_d = nc.dram_tensor("out", [RROWS, E], F32, kind="ExternalOutput")

    RG = [list(range(NCORES))]

    with tile.TileContext(nc) as tc:
        with (
            tc.tile_pool(name="persist", bufs=1) as pp,
            tc.tile_pool(name="dram", bufs=1, space="DRAM") as dp,
            tc.tile_pool(name="ps512", bufs=4, space="PSUM") as ps512,
            tc.tile_pool(name="psT", bufs=2, space="PSUM") as psT,
            tc.tile_pool(name="psR", bufs=2, space="PSUM") as psR,
        ):
            # ---- collective bounce buffers (internal DRAM) ----
            att_in = [
                [
                    [
                        dp.tile([S, 512], F16, name=f"att_in_{h}_{fh}_{fb}", tag=f"ati{h}{fh}{fb}")
                        for fb in range(2)
                    ]
                    for fh in range(2)
                ]
                for h in range(HPC)
            ]
            att_out = [
                [
                    [
                        dp.tile(
                            [RROWS, 512],
                            F16,
                            name=f"att_out_{h}_{fh}_{fb}",
                            tag=f"ato{h}{fh}{fb}",
                        )
                        for fb in range(2)
                    ]
                    for fh in range(2)
                ]
                for h in range(HPC)
            ]
            st1_in = dp.tile([1, 8], F32, name="st1_in", tag="st1i")
            st1_out = dp.tile([1, 8], F32, name="st1_out", tag="st1o", addr_space="Shared")
            st2_in = dp.tile([1, 8], F32, name="st2_in", tag="st2i")
            st2_out = dp.tile([1, 8], F32, name="st2_out", tag="st2o", addr_space="Shared")
            agt_in = [
                dp.tile([E // 2, RROWS], F16, name=f"agt_in{j}", tag=f"agi{j}")
                for j in range(2)
            ]
            agt_out = [
                dp.tile(
                    [NCORES * (E // 2), RROWS],
                    F16,
                    name=f"agt_out{j}",
                    tag=f"ago{j}",
                    addr_space="Shared",
                )
                for j in range(2)
            ]
            ffn_in = [
                dp.tile([S, 512], F16, name=f"ffn_in_{eb}", tag=f"ffi{eb}")
                for eb in range(4)
            ]
            ffn_out = [
                dp.tile([RROWS, 512], F16, name=f"ffn_out_{eb}", tag=f"ffo{eb}")
                for eb in range(4)
            ]

            # ---- persistent small tiles ----
            xh = pp.tile([128, 16, S], BF16, name="xh")  # x^T; becomes h^T in place
            onesc = pp.tile([128, 8], F32, name="onesc")
            nc.sync.dma_start(onesc[:], ones_d[:])
            onesr = pp.tile([1, 128], F32, name="onesr")
            nc.sync.dma_start(onesr[:], onesr_d[:])
            ones_bf = pp.tile([128, 1], BF16, name="ones_bf")
            nc.vector.tensor_copy(out=ones_bf[:], in_=onesc[:, 0:1])
            bq_sb = pp.tile([128, HPC], F32, name="bq_sb")
            nc.sync.dma_start(bq_sb[:], bq_d[:])
            bk_sb = pp.tile([128, HPC], F32, name="bk_sb")
            nc.sync.dma_start(bk_sb[:], bk_d[:])
            b1_sb = pp.tile([128, 8], F32, name="b1_sb")
            nc.sync.dma_start(b1_sb[:], b1_d[:])
            yb1T = pp.tile([128, 16], F32, name="yb1T")
            nc.sync.dma_start(yb1T[:], yb1T_d[:])
            recips = pp.tile([128, HPC, 16], F32, name="recips")
            bc1 = pp.tile([128, 2], F32, name="bc1")

            with tc.tile_pool(name="attn", bufs=1) as ap_:
                maskb = ap_.tile([128, 4, 512], BF16, name="maskb")
                eT = ap_.tile([128, NTRI, 512], BF16, name="eT")
                v_sb = ap_.tile([128, 16, 512], BF16, name="v_sb")

                with (
                    tc.tile_pool(name="wvb", bufs=2) as wvbp,
                    tc.tile_pool(name="astg", bufs=4) as astg,
                ):

                    def _vproj(h, fh, fb):
                        with nc.named_scope(f"vproj{h}{fh}{fb}"):
                            wvb = wvbp.tile([128, 16, 512], BF16, name="wvb", tag="wvb")
                            nc.sync.dma_start(
                                wvb[:],
                                wv_d[
                                    h, :, :,
                                    fh * 1024 + fb * 512 : fh * 1024 + (fb + 1) * 512,
                                ],
                            )
                            for tcn in range(16):
                                pv = ps512.tile([128, 512], F32, name="pv", tag="p512")
                                for eo in range(16):
                                    nc.tensor.matmul(
                                        pv[:],
                                        xh[:, eo, tcn * 128 : (tcn + 1) * 128],
                                        wvb[:, eo, :],
                                        start=(eo == 0),
                                        stop=(eo == 15),
                                    )
                                nc.vector.tensor_copy(out=v_sb[:, tcn, :], in_=pv[:])

                    def _attnv(h, fh, fb):
                        with nc.named_scope(f"attnv{h}{fh}{fb}"):
                            do_r = fh == 0 and fb == 0
                            for i in range(15, -1, -1):
                                sb, so = i // 4, (i % 4) * 128
                                pa = ps512.tile([128, 512], F32, name="pa", tag="p512")
                                if do_r:
                                    pr = psR.tile([128, 1], F32, name="pr", tag="pr")
                                for tcn in range(i + 1):
                                    lhs = eT[:, TRI[sb] + tcn, so : so + 128]
                                    nc.tensor.matmul(
                                        pa[:],
                                        lhs,
                                        v_sb[:, tcn, :],
                                        start=(tcn == 0),
                                        stop=(tcn == i),
                                    )
                                    if do_r:
                                        nc.tensor.matmul(
                                            pr[:],
                                            lhs,
                                            ones_bf[:],
                                            start=(tcn == 0),
                                            stop=(tcn == i),
                                        )
                                if do_r:
                                    rsf = astg.tile([128, 1], F32, name="rsf", tag="rsf")
                                    nc.vector.tensor_copy(out=rsf[:], in_=pr[:])
                                    nc.vector.reciprocal(recips[:, h, i : i + 1], rsf[:])
                                stg = astg.tile([128, 512], F16, name="stg", tag="stg")
                                nc.scalar.activation(
                                    stg[:],
                                    pa[:],
                                    AF.Copy,
                                    scale=recips[:, h, i : i + 1],
                                )
                                nc.sync.dma_start(
                                    att_in[h][fh][fb][i * 128 : (i + 1) * 128, :],
                                    stg[:],
                                )
                            nc.gpsimd.collective_compute(
                                "ReduceScatter",
                                AL.add,
                                replica_groups=RG,
                                ins=[att_in[h][fh][fb][:]],
                                outs=[att_out[h][fh][fb][:]],
                            )

                    def _scores(h):
                        with nc.named_scope(f"scores{h}"):
                            for sb in range(4):
                                for tcn in range(4 * sb + 4):
                                    psc = ps512.tile([128, 512], F32, name="psc", tag="p512")
                                    nc.tensor.matmul(
                                        psc[:],
                                        qkT[:, 1, h, tcn * 128 : (tcn + 1) * 128],
                                        qkT[:, 0, h, sb * 512 : (sb + 1) * 512],
                                        start=True,
                                        stop=True,
                                    )
                                    dst = eT[:, TRI[sb] + tcn, :]
                                    if tcn >= 4 * sb:
                                        etmp = astg.tile(
                                            [128, 512], BF16, name="etmp", tag="etmp", bufs=3
                                        )
                                        nc.scalar.activation(etmp[:], psc[:], AF.Exp)
                                        nc.vector.tensor_tensor(
                                            dst, etmp[:], maskb[:, tcn - 4 * sb, :], AL.mult
                                        )
                                    else:
                                        nc.scalar.activation(dst, psc[:], AF.Exp)

                    def _midwork(fh):
                        with nc.named_scope(f"mid{fh}"):
                            aTo = mw.tile(
                                [128, 8, RROWS], F16, name="aTo", tag="aTo", bufs=2
                            )
                            for rt in range(2):
                                for h in range(2):
                                    for fb in range(2):
                                        nc.sync.dma_start(
                                            rof[:, rt, h, fb * 512 : (fb + 1) * 512],
                                            att_out[h][fh][fb][
                                                rt * 128 : (rt + 1) * 128, :
                                            ],
                                        )
                                nc.vector.tensor_tensor(
                                    rsum[:, rt, :], rof[:, rt, 0, :], rof[:, rt, 1, :], AL.add
                                )
                                nc.sync.dma_start(
                                    aTo[:, :, rt * 128 : (rt + 1) * 128],
                                    rsum[:, rt, :],
                                    transpose=True,
                                )
                            nc.sync.dma_start(
                                agt_in[fh].rearrange("(ec p) s -> p ec s", p=128),
                                aTo[:],
                            )
                            if fh == 0:
                                nc.gpsimd.collective_compute(
                                    "AllGather",
                                    AL.bypass,
                                    replica_groups=RG,
                                    ins=[agt_in[0][:]],
                                    outs=[agt_out[0][:]],
                                )
                            # LN1 stats partial from transposed own shard
                            nc.vector.tensor_tensor(
                                yo[:], xg[:, fh * 8 : (fh + 1) * 8, :], aTo[:], AL.add
                            )
                            yof = yo.rearrange("p a b -> p (a b)")
                            nc.vector.tensor_reduce(
                                parts[:, fh : fh + 1], yof, axis=AX.X, op=AL.add
                            )
                            nc.scalar.activation(
                                sqs2[:], yof, AF.Square,
                                accum_out=parts[:, 2 + fh : 3 + fh],
                            )

                    with tc.tile_pool(name="qkp", bufs=1) as qkp:
                        qkT = qkp.tile([128, 2, HPC, S], BF16, name="qkT")
                        with tc.tile_pool(name="qkw", bufs=1) as qkw:
                            wqk = [
                                [
                                    qkw.tile(
                                        [128, 16, KD], BF16,
                                        name=f"wqk{h}{qi}", tag=f"wqk{h}{qi}",
                                    )
                                    for qi in range(2)
                                ]
                                for h in range(HPC)
                            ]
                            for h in range(HPC):
                                nc.sync.dma_start(wqk[h][0][:], wq_d[h])
                                nc.sync.dma_start(wqk[h][1][:], wk_d[h])
                            for eo in range(16):
                                nc.sync.dma_start(xh[:, eo, :], xtb_d[:, eo, :])
                            nc.sync.dma_start(maskb[:], mask_d[:])
                            with tc.tile_pool(name="prep", bufs=1) as prep, nc.named_scope("prep"):
                                wtile = prep.tile([128, 512], BF16, name="wtile", tag="wtile")
                                nc.vector.memset(wtile[:], 0.0)
                                for _w in range(24):
                                    pw = ps512.tile([128, 512], F32, name="pw", tag="p512")
                                    nc.tensor.matmul(
                                        pw[:], wtile[:, :128], wtile[:], start=True, stop=True
                                    )
                            with nc.named_scope("qkproj"):
                                for h in range(HPC):
                                    for qi, (b_sb, scl) in enumerate(
                                        ((bq_sb, ISCALE), (bk_sb, 1.0))
                                    ):
                                        wb = wqk[h][qi]
                                        pqs = [
                                            ps512.tile([128, 512], F32, name=f"pq{sb}", tag="p512")
                                            for sb in range(4)
                                        ]
                                        for eo in range(16):
                                            for sb in range(4):
                                                nc.tensor.matmul(
                                                    pqs[sb][:],
                                                    wb[:, eo, :],
                                                    xh[:, eo, sb * 512 : (sb + 1) * 512],
                                                    start=(eo == 0),
                                                    stop=(eo == 15),
                                                )
                                        for sb in range(4):
                                            nc.scalar.activation(
                                                qkT[:, qi, h, sb * 512 : (sb + 1) * 512],
                                                pqs[sb][:],
                                                AF.Identity,
                                                bias=b_sb[:, h : h + 1],
                                                scale=scl,
                                            )
                        # head 0 fully inside qkp (eT reused by head 1)
                        _scores(0)
                        for fh in range(2):
                            for fb in range(2):
                                _vproj(0, fh, fb)
                                _attnv(0, fh, fb)
                        _scores(1)
                    # qkp closed: its space is reused by the midwork pool
                    with tc.tile_pool(name="mw", bufs=1) as mw:
                      xg = mw.tile([128, 16, RROWS], BF16, name="xg")
                      nc.sync.dma_start(xg[:], xg_d[:])
                      rof = mw.tile([128, 2, 2, FSH], F16, name="rof")
                      rsum = mw.tile([128, 2, FSH], F16, name="rsum")
                      yo = mw.tile([128, 8, RROWS], BF16, name="yo")
                      sqs2 = mw.tile([128, 8 * RROWS], BF16, name="sqs2")
                      parts = mw.tile([128, 4], F32, name="parts")
                      stat1 = mw.tile([1, 16], F32, name="stat1")
                      for fb in range(2):
                        _vproj(1, 0, fb)
                        _attnv(1, 0, fb)
                      _midwork(0)
                      for fb in range(2):
                        _vproj(1, 1, fb)
                        _attnv(1, 1, fb)
                      _midwork(1)
                      # LN1 stats AllReduce (before the 2nd AllGather on the
                      # collective queue)
                      with nc.named_scope("ln1ar"):
                        pstat = psT.tile([128, 128], F32, name="pstat", tag="pt")
                        nc.tensor.matmul(
                            pstat[:1, :4], onesc[:, 0:1], parts[:], start=True, stop=True
                        )
                        nc.vector.tensor_copy(out=stat1[:, 0:4], in_=pstat[:1, :4])
                        nc.vector.memset(stat1[:, 8:16], 0.0)
                        nc.vector.tensor_reduce(
                            stat1[:, 8:9], stat1[:, 0:2], axis=AX.X, op=AL.add
                        )
                        nc.vector.tensor_reduce(
                            stat1[:, 9:10], stat1[:, 2:4], axis=AX.X, op=AL.add
                        )
                        nc.sync.dma_start(st1_in[:], stat1[:, 8:16])
                        nc.gpsimd.collective_compute(
                            "AllReduce", AL.add, replica_groups=RG,
                            ins=[st1_in[:]], outs=[st1_out[:]],
                        )
                      nc.gpsimd.collective_compute(
                          "AllGather",
                          AL.bypass,
                          replica_groups=RG,
                          ins=[agt_in[1][:]],
                          outs=[agt_out[1][:]],
                      )
                      with nc.named_scope("ln1sc"):
                          bcv = _ln_scalars(nc, mw, psT, onesr, st1_out)
                          nc.vector.tensor_copy(out=bc1[:], in_=bcv[:])

            # =========== LN1 affine over full y^T, in place into xh ===========
            with tc.tile_pool(name="aff", bufs=1) as gb, nc.named_scope("affine"):
                bias16 = gb.tile([128, 16], F32, name="bias16")
                # bias per (e-partition, eo): (yb1T - m) * rstd
                nc.scalar.activation(
                    bias16[:], yb1T[:], AF.Identity, bias=bc1[:, 0:1], scale=bc1[:, 1:2]
                )
                for eo in range(16):
                    fh, el = eo // 8, eo % 8
                    aTe = gb.tile([128, S], F16, name="aTe", tag="aTe", bufs=3)
                    for c in range(NCORES):
                        nc.sync.dma_start(
                            aTe[:, c * RROWS : (c + 1) * RROWS],
                            agt_out[fh][
                                c * (E // 2) + el * 128 : c * (E // 2) + (el + 1) * 128, :
                            ],
                        )
                    gch = gb.tile([128, S], BF16, name="gch", tag="gch", bufs=3)
                    nc.sync.dma_start(gch[:], gT_d[:, eo, :])
                    bch = gb.tile([128, S], BF16, name="bch", tag="bch", bufs=3)
                    nc.sync.dma_start(bch[:], bT_d[:, eo, :])
                    t1 = gb.tile([128, S], BF16, name="t1", tag="t1", bufs=2)
                    nc.vector.tensor_tensor(t1[:], xh[:, eo, :], aTe[:], AL.add)
                    t2 = gb.tile([128, S], BF16, name="t2", tag="t2", bufs=2)
                    nc.scalar.activation(
                        t2[:], t1[:], AF.Identity,
                        bias=bias16[:, eo : eo + 1], scale=bc1[:, 1:2],
                    )
                    nc.vector.tensor_tensor(t2[:], t2[:], gch[:], AL.mult)
                    nc.vector.tensor_tensor(xh[:, eo, :], t2[:], bch[:], AL.add)

            # =========== FFN + rowwise h_own + LN2 ===========
            with tc.tile_pool(name="ffn", bufs=1) as fp:
                zT = fp.tile([128, 8, S], BF16, name="zT")
                h_own = fp.tile([128, 2, E], BF16, name="h_own")
                ys = fp.tile([128, 2, E], F32, name="ys")
                lngt = fp.tile([128, 2, E], BF16, name="lngt")
                lnbt = fp.tile([128, 2, E], BF16, name="lnbt")

                with nc.named_scope("ffn1"), tc.tile_pool(name="wst", bufs=2) as wst:
                    for ft in range(8):
                        w1b = wst.tile([128, 2048], BF16, name="w1b", tag="w1b")
                        nc.sync.dma_start(w1b[:], w1_d[:, ft, :])
                        pzs = [
                            ps512.tile([128, 512], F32, name=f"pz{sb}", tag="p512")
                            for sb in range(4)
                        ]
                        for eo in range(16):
                            for sb in range(4):
                                nc.tensor.matmul(
                                    pzs[sb][:],
                                    w1b[:, eo * 128 : (eo + 1) * 128],
                                    xh[:, eo, sb * 512 : (sb + 1) * 512],
                                    start=(eo == 0),
                                    stop=(eo == 15),
                                )
                        for sb in range(4):
                            nc.scalar.activation(
                                zT[:, ft, sb * 512 : (sb + 1) * 512],
                                pzs[sb][:],
                                AF.Relu,
                                bias=b1_sb[:, ft : ft + 1],
                            )

                # rowwise y rebuild -> h_own (off the critical path; needs bc1)
                with nc.named_scope("hown"), tc.tile_pool(name="hop", bufs=1) as hp:
                    yb1t = hp.tile([128, E], F32, name="yb1t")
                    nc.sync.dma_start(yb1t[:], yb1_d[:])
                    nc.sync.dma_start(lngt[:], lng_d.ap().rearrange("(t p) e -> p t e", p=128))
                    nc.sync.dma_start(lnbt[:], lnb_d.ap().rearrange("(t p) e -> p t e", p=128))
                    for rt in range(2):
                        xrt = hp.tile([128, E], F32, name="xrt", tag="xrt")
                        nc.sync.dma_start(xrt[:], xr_d[rt * 128 : (rt + 1) * 128, :])
                        nc.vector.tensor_tensor(ys[:, rt, :], xrt[:], yb1t[:], AL.add)
                        for fh in range(2):
                            dstv = ys[:, rt, fh * FSH : (fh + 1) * FSH]
                            for h in range(2):
                                rof2 = hp.tile([128, FSH], F16, name="rof2", tag="rof2", bufs=2)
                                for fb in range(2):
                                    nc.sync.dma_start(
                                        rof2[:, fb * 512 : (fb + 1) * 512],
                                        att_out[h][fh][fb][rt * 128 : (rt + 1) * 128, :],
                                    )
                                nc.vector.tensor_tensor(dstv, dstv, rof2[:], AL.add)
                        ht = hp.tile([128, E], F32, name="ht", tag="ht")
                        nc.scalar.activation(
                            ht[:], ys[:, rt, :], AF.Identity,
                            bias=bc1[:, 0:1], scale=bc1[:, 1:2],
                        )
                        nc.vector.tensor_tensor(ht[:], ht[:], lngt[:, rt, :], AL.mult)
                        nc.vector.tensor_tensor(h_own[:, rt, :], ht[:], lnbt[:, rt, :], AL.add)

                with nc.named_scope("ffn2"), tc.tile_pool(name="w2p", bufs=2) as w2p:
                    for eb in range(4):
                        w2b = w2p.tile([128, 8, 512], BF16, name="w2b", tag="w2b")
                        nc.sync.dma_start(w2b[:], w2_d[:, :, eb, :])
                        for i in range(15, -1, -1):
                            pf = ps512.tile([128, 512], F32, name="pf", tag="p512")
                            for fc in range(8):
                                nc.tensor.matmul(
                                    pf[:],
                                    zT[:, fc, i * 128 : (i + 1) * 128],
                                    w2b[:, fc, :],
                                    start=(fc == 0),
                                    stop=(fc == 7),
                                )
                            fstg = w2p.tile([128, 512], F16, name="fstg", tag="fstg", bufs=4)
                            nc.scalar.activation(fstg[:], pf[:], AF.Copy)
                            nc.sync.dma_start(
                                ffn_in[eb][i * 128 : (i + 1) * 128, :], fstg[:]
                            )
                        nc.gpsimd.collective_compute(
                            "ReduceScatter",
                            AL.add,
                            replica_groups=RG,
                            ins=[ffn_in[eb][:]],
                            outs=[ffn_out[eb][:]],
                        )

                # =========== LN2 + output ===========
                with tc.tile_pool(name="ln2", bufs=1) as l2, nc.named_scope("ln2"):
                    yb2t = l2.tile([128, E], F32, name="yb2t")
                    nc.sync.dma_start(yb2t[:], yb2_d[:])
                    for rt in range(2):
                        nc.vector.tensor_tensor(
                            ys[:, rt, :], h_own[:, rt, :], yb2t[:], AL.add
                        )
                        for eb in range(4):
                            fot = l2.tile([128, 512], F16, name="fot", tag="fot", bufs=2)
                            nc.sync.dma_start(
                                fot[:], ffn_out[eb][rt * 128 : (rt + 1) * 128, :]
                            )
                            dstv = ys[:, rt, eb * 512 : (eb + 1) * 512]
                            nc.vector.tensor_tensor(dstv, dstv, fot[:], AL.add)

                    _stats_ln(nc, tc, l2, psT, ys, onesc, onesr, st2_in, st2_out, RG)
                    bc2 = _ln_scalars(nc, l2, psT, onesr, st2_out)
                    for rt in range(2):
                        ot = l2.tile([128, E], F32, name="ot", tag="ot", bufs=2)
                        nc.scalar.activation(
                            ot[:],
                            ys[:, rt, :],
                            AF.Identity,
                            bias=bc2[:, 0:1],
                            scale=bc2[:, 1:2],
                        )
                        nc.vector.tensor_tensor(ot[:], ot[:], lngt[:, rt, :], AL.mult)
                        nc.vector.tensor_tensor(ot[:], ot[:], lnbt[:, rt, :], AL.add)
                        nc.sync.dma_start(out_d[rt * 128 : (rt + 1) * 128, :], ot[:])

    nc.compile()
    return nc


def _stats_ln(nc, tc, pool, psT, ys, onesc, onesr, st_in, st_out, RG):
    """partial sum/sumsq of ys [128, 2, E] -> tiny fp32 AllReduce."""
    parts = pool.tile([128, 8], F32, name="parts", tag="parts")
    sqs = pool.tile([128, E // 2], BF16, name="sqs", tag="sqs")
    for rt in range(2):
        for ch in range(2):
            idx = rt * 2 + ch
            ysl = ys[:, rt, ch * (E // 2) : (ch + 1) * (E // 2)]
            nc.vector.tensor_reduce(parts[:, idx : idx + 1], ysl, axis=AX.X, op=AL.add)
            nc.scalar.activation(
                sqs[:], ysl, AF.Square, accum_out=parts[:, 4 + idx : 5 + idx]
            )
    pstat = psT.tile([128, 128], F32, name="pstat", tag="pt")
    nc.tensor.matmul(pstat[:1, :8], onesc[:, 0:1], parts[:], start=True, stop=True)
    st4s = pool.tile([1, 8], F32, name="st4s", tag="st4s")
    nc.vector.tensor_copy(out=st4s[:], in_=pstat[:1, :8])
    st4 = pool.tile([1, 8], F32, name="st4", tag="st4")
    nc.vector.memset(st4[:], 0.0)
    nc.vector.tensor_reduce(st4[:, 0:1], st4s[:, 0:4], axis=AX.X, op=AL.add)
    nc.vector.tensor_reduce(st4[:, 1:2], st4s[:, 4:8], axis=AX.X, op=AL.add)
    nc.sync.dma_start(st_in[:], st4[:])
    nc.gpsimd.collective_compute(
        "AllReduce", AL.add, replica_groups=RG, ins=[st_in[:]], outs=[st_out[:]]
    )


def _ln_scalars(nc, pool, psT, onesr, st_out):
    """AllReduced (sum, sumsq) -> bc [128, 2] = (-m*rstd, rstd) broadcast."""
    so = pool.tile([1, 8], F32, name="so", tag="so")
    nc.sync.dma_start(so[:], st_out[:])
    sc = pool.tile([1, 8], F32, name="sc", tag="sc")
    nc.scalar.mul(sc[:, 0:1], so[:, 0:1], 1.0 / NTOT)
    nc.scalar.mul(sc[:, 1:2], so[:, 1:2], 1.0 / NTOT)
    nc.scalar.activation(sc[:, 2:3], sc[:, 0:1], AF.Square)
    nc.vector.tensor_tensor(sc[:, 3:4], sc[:, 1:2], sc[:, 2:3], AL.subtract)
    nc.vector.tensor_scalar_add(sc[:, 2:3], sc[:, 3:4], EPS)  # var + eps
    nc.scalar.activation(sc[:, 6:7], sc[:, 2:3], AF.Ln)
    nc.scalar.activation(sc[:, 4:5], sc[:, 6:7], AF.Exp, scale=-0.5)
    nc.vector.tensor_tensor(sc[:, 7:8], sc[:, 0:1], sc[:, 4:5], AL.mult)
    nc.scalar.mul(sc[:, 5:6], sc[:, 7:8], -1.0)
    s2 = pool.tile([1, 2], F32, name="s2", tag="s2")
    nc.vector.tensor_copy(out=s2[:, 0:1], in_=sc[:, 5:6])
    nc.vector.tensor_copy(out=s2[:, 1:2], in_=sc[:, 4:5])
    pb = psT.tile([128, 128], F32, name="pb", tag="pt")
    nc.tensor.matmul(pb[:, :2], onesr[:], s2[:], start=True, stop=True)
    bc = pool.tile([128, 2], F32, name="bc", tag="bc")
    nc.vector.tensor_copy(out=bc[:], in_=pb[:, :2])
    return bc


_NC_CACHE = None


def _get_nc():
    global _NC_CACHE
    if _NC_CACHE is None:
        _NC_CACHE = _build()
    return _NC_CACHE


def _shared_prep(inputs):
    """Host tensors identical across cores (built once)."""
    f32 = np.float32
    x = np.ascontiguousarray(inputs["input"], dtype=f32)
    ln_g, ln_b = inputs["ln_g"], inputs["ln_b"]
    bv, b2 = inputs["bv"], inputs["b2"]
    jj, tp, sf = np.meshgrid(
        np.arange(4), np.arange(128), np.arange(512), indexing="ij"
    )
    mask = ((128 * jj + tp) <= sf).astype(_bf16)
    xtb = np.ascontiguousarray(
        x.T.reshape(16, 128, S).transpose(1, 0, 2).astype(_bf16)
    )
    gT = np.ascontiguousarray(
        np.asarray(ln_g, f32).T.reshape(16, 128, S).transpose(1, 0, 2).astype(_bf16)
    )
    bT = np.ascontiguousarray(
        np.asarray(ln_b, f32).T.reshape(16, 128, S).transpose(1, 0, 2).astype(_bf16)
    )
    bvsum = np.asarray(bv, f32).sum(axis=0)
    return {
        "x": x,
        "xtb": xtb,
        "gT": gT,
        "bT": bT,
        "mask": np.ascontiguousarray(mask.transpose(1, 0, 2)),
        "ones": np.ones((128, 8), dtype=f32),
        "onesr": np.ones((1, 128), dtype=f32),
        "yb1": np.ascontiguousarray(np.broadcast_to(bvsum, (128, E)), dtype=f32),
        "yb1T": np.ascontiguousarray(bvsum.reshape(16, 128).T, dtype=f32),
        "yb2": np.ascontiguousarray(
            np.broadcast_to(np.asarray(b2, f32), (128, E)), dtype=f32
        ),
        "bvsum": bvsum,
    }


def _prep_core(c, inputs, shared=None):
    f32 = np.float32
    if shared is None:
        shared = _shared_prep(inputs)
    x = shared["x"]
    Wq, Wk, Wv = inputs["Wq"], inputs["Wk"], inputs["Wv"]
    bq, bk = inputs["bq"], inputs["bk"]
    W1, b1, W2 = inputs["W1"], inputs["b1"], inputs["W2"]
    ln_g, ln_b = inputs["ln_g"], inputs["ln_b"]
    h0 = c * HPC
    wqt = np.ascontiguousarray(
        np.stack(
            [Wq[h0 + h].reshape(16, 128, KD).transpose(1, 0, 2) for h in range(HPC)]
        ).astype(_bf16)
    )
    wkt = np.ascontiguousarray(
        np.stack(
            [Wk[h0 + h].reshape(16, 128, KD).transpose(1, 0, 2) for h in range(HPC)]
        ).astype(_bf16)
    )
    wvt = np.ascontiguousarray(
        np.stack(
            [Wv[h0 + h].reshape(16, 128, E).transpose(1, 0, 2) for h in range(HPC)]
        ).astype(_bf16)
    )
    W1s = np.asarray(W1)[:, c * FSH : (c + 1) * FSH]
    w1t = np.ascontiguousarray(
        W1s.reshape(16, 128, 8, 128).transpose(1, 2, 0, 3).reshape(128, 8, 2048).astype(_bf16)
    )
    W2s = np.asarray(W2)[c * FSH : (c + 1) * FSH, :]
    w2t = np.ascontiguousarray(
        W2s.reshape(8, 128, 4, 512).transpose(1, 0, 2, 3).astype(_bf16)
    )
    bqs = np.ascontiguousarray((np.asarray(bq, f32)[h0 : h0 + HPC] * ISCALE).T, dtype=f32)
    bks = np.ascontiguousarray(np.asarray(bk, f32)[h0 : h0 + HPC].T, dtype=f32)
    b1s = np.ascontiguousarray(
        np.asarray(b1, f32)[c * FSH : (c + 1) * FSH].reshape(8, 128).T, dtype=f32
    )
    ridx = _rows_idx(c)
    xo = x[ridx] + shared["bvsum"][None, :]
    xg = np.ascontiguousarray(
        xo.T.reshape(16, 128, RROWS).transpose(1, 0, 2).astype(_bf16)
    )
    return {
        "xtb": shared["xtb"],
        "gT": shared["gT"],
        "bT": shared["bT"],
        "mask": shared["mask"],
        "ones": shared["ones"],
        "onesr": shared["onesr"],
        "yb1": shared["yb1"],
        "yb1T": shared["yb1T"],
        "yb2": shared["yb2"],
        "wqt": wqt,
        "wkt": wkt,
        "wvt": wvt,
        "w1t": w1t,
        "w2t": w2t,
        "bqs": bqs,
        "bks": bks,
        "b1s": b1s,
        "xr": np.ascontiguousarray(x[ridx], dtype=f32),
        "xg": xg,
        "lngr": np.ascontiguousarray(np.asarray(ln_g, f32)[ridx].astype(_bf16)),
        "lnbr": np.ascontiguousarray(np.asarray(ln_b, f32)[ridx].astype(_bf16)),
    }


def _assemble(results):
    out = np.empty((S, E), dtype=np.float32)
    for c in range(NCORES):
        out[_rows_idx(c)] = results[c]["out"]
    return out


def kernel(**inputs):
    nc = _get_nc()
    inputs = {k: np.asarray(v, dtype=np.float32) for k, v in inputs.items()}
    shared = _shared_prep(inputs)
    in_maps = [_prep_core(c, inputs, shared) for c in range(NCORES)]
    res = run_bass_kernel_spmd(nc, in_maps, core_ids=list(range(NCORES)))
    return np.ascontiguousarray(_assemble(res.results), dtype=np.float32)
